# revision 9
# baseline (speedup 1.0000x reference)
"""Fp8 per-token/per-channel quantized linear for Trainium2, 8 NeuronCores.

Computation (matches the jax reference):
    amax[m]  = max_k |x[m, k]|                       (x is bf16)
    xs[m]    = max(amax, 1e-10) / 448
    x_q      = e4m3fn_round(x / xs)                  (values up to +-448)
    out      = bf16((x_q @ W^T) * xs * w_scales) + bf16(bias)

Mapping to TRN2 hardware:
  * TRN's fp8 E4M3 saturates at +-240 (256..448 are Inf/NaN), so we quantize
    at HALF scale: x_q' = e4m3_round(x * (224/amax)) == x_q / 2 exactly (the
    fp8 grid is self-similar under powers of two), and fold the factor 2 into
    the output scale: out = psum * (amax/224) * w_scales.  The reference
    weights are already exactly fp8-representable, so casting them is lossless.
  * Sharding: row-parallel over M (8 cores x 1024 rows).  Each core quantizes
    only its own rows (the amax reduction is the expensive vector-engine op;
    replicating it 8x under column-parallel would make the kernel DVE-bound),
    and streams the full weight, transposed on host to [K, N] tile layout,
    cast fp32->fp8 in-flight by the SWDGE DMA.
  * x_q is transposed on-chip into [K, M] layout with PE identity matmuls
    (contraction must sit on partitions for both matmul operands).
  * Main GEMM runs in fp8 with perf_mode=DoubleRow (k=256 per matmul).
"""

import os
import numpy as np
import ml_dtypes
from contextlib import ExitStack

import concourse.bass as bass
import concourse.bacc as bacc
import concourse.tile as tile
from concourse import mybir
from concourse.bass_utils import run_bass_kernel_spmd
from concourse.masks import make_identity

P = 128
M, K, N = 8192, 4096, 4096
NCORES = 8
M_SHARD = M // NCORES          # 1024 rows of x per core
M_TILES = M_SHARD // P         # 8
K_SUBS = K // P                # 32
K_SUPERS = K // (2 * P)        # 16 (DoubleRow consumes 256 rows of K)
N_BLK = 512
N_BLKS = N // N_BLK            # 8

FP8 = mybir.dt.float8e4
F32 = mybir.dt.float32
BF16 = mybir.dt.bfloat16

_PROGRAM_CACHE = {}


def _build_program():
    nc = bacc.Bacc(None, target_bir_lowering=False)

    x_d = nc.declare_dram_parameter("x", [M_SHARD, K], BF16, isOutput=False)
    # host layout: wt[nb, p, ksub, n] = weight[nb*512 + n, ksub*128 + p],
    # losslessly re-encoded to fp8 (reference weights are fp8-round-tripped,
    # i.e. every value is exactly representable in e4m3)
    wt_d = nc.declare_dram_parameter("wt", [N_BLKS, P, K_SUBS, N_BLK], FP8, isOutput=False)
    ws_d = nc.declare_dram_parameter("ws", [N], F32, isOutput=False)
    bias_d = nc.declare_dram_parameter("bias", [N], F32, isOutput=False)
    out_d = nc.declare_dram_parameter("out", [M_SHARD, N], BF16, isOutput=True)

    x_ap = x_d[:]
    wt_ap = wt_d[:]
    out_ap = out_d[:]

    with tile.TileContext(nc) as tc, ExitStack() as ctx:
        singles = ctx.enter_context(tc.tile_pool(name="singles", bufs=1))
        xpool = ctx.enter_context(tc.tile_pool(name="xpool", bufs=4))
        stats = ctx.enter_context(tc.tile_pool(name="stats", bufs=4))
        xspool = ctx.enter_context(tc.tile_pool(name="xspool", bufs=M_TILES))
        xqtpool = ctx.enter_context(tc.tile_pool(name="xqtpool", bufs=M_TILES))
        wpool = ctx.enter_context(tc.tile_pool(name="wpool", bufs=3))
        opool = ctx.enter_context(tc.tile_pool(name="opool", bufs=4))
        psum_tr = ctx.enter_context(tc.tile_pool(name="psum_tr", bufs=2, space="PSUM"))
        psum_mm = ctx.enter_context(tc.tile_pool(name="psum_mm", bufs=4, space="PSUM"))

        # w-scale / bias broadcasts ride the ACT HWDGE ring so they don't
        # delay the x loads (sync ring) or the w slabs (gpsimd/SWDGE ring)
        ws_b = singles.tile([P, N], F32)
        nc.scalar.dma_start(
            out=ws_b[:],
            in_=bass.AP(tensor=ws_d[:].tensor, offset=0, ap=[[0, P], [1, N]]),
        )
        bias_f32 = singles.tile([P, N], F32)
        nc.scalar.dma_start(
            out=bias_f32[:],
            in_=bass.AP(tensor=bias_d[:].tensor, offset=0, ap=[[0, P], [1, N]]),
        )

        # prefetch the first weight slabs before anything else on gpsimd
        wslab_tiles = [None] * N_BLKS

        def issue_wslab(nb):
            t = wpool.tile([P, K_SUBS, N_BLK], FP8, tag="w")
            nc.scalar.dma_start(out=t[:], in_=wt_ap[nb])
            wslab_tiles[nb] = t

        issue_wslab(0)
        issue_wslab(1)
        issue_wslab(2)

        ident = singles.tile([P, P], FP8)
        make_identity(nc, ident)

        bias_b = singles.tile([P, N], BF16)
        nc.gpsimd.tensor_copy(out=bias_b[:], in_=bias_f32[:])

        # ---- quantization phase: per 128-row tile of x ----
        xs_tiles = []
        xqt_tiles = []
        for mt in range(M_TILES):
            xt = xpool.tile([P, K], BF16, tag="xt")
            nc.sync.dma_start(out=xt[:], in_=x_ap[mt * P:(mt + 1) * P, :])

            amax = stats.tile([P, 1], F32, tag="amax")
            nc.vector.tensor_reduce(
                out=amax[:], in_=xt[:],
                axis=mybir.AxisListType.X, op=mybir.AluOpType.max,
                apply_absolute_value=True,
            )
            # keep the short per-tile chain ahead of later tiles' reduces so
            # the first quantized tile reaches the PE as early as possible
            with tc.high_priority():
                nc.vector.tensor_scalar_max(out=amax[:], in0=amax[:], scalar1=1e-10)
                inv = stats.tile([P, 1], F32, tag="inv")
                nc.vector.reciprocal(out=inv[:], in_=amax[:])
                nc.vector.tensor_scalar_mul(out=inv[:], in0=inv[:], scalar1=224.0)
                xs = xspool.tile([P, 1], F32, tag="xs")
                nc.vector.tensor_scalar_mul(out=xs[:], in0=amax[:], scalar1=1.0 / 224.0)
                xs_tiles.append(xs)

                xq = xpool.tile([P, K], FP8, tag="xq")
                nc.scalar.activation(
                    out=xq[:], in_=xt[:],
                    func=mybir.ActivationFunctionType.Copy, scale=inv[:],
                )

            # transpose x_q into [K, M] layout via PE identity matmuls
            xqt = xqtpool.tile([P, K_SUBS, P], FP8, tag="xqt")
            xqt_tiles.append(xqt)
            for q8 in range(K_SUBS // 8):
                ptr = psum_tr.tile([P, 8, P], F32, tag="ptr")
                for i in range(8):
                    ks = q8 * 8 + i
                    nc.tensor.matmul(
                        out=ptr[:, i, :],
                        lhsT=xq[:, ks * P:(ks + 1) * P],
                        rhs=ident[:],
                        start=True, stop=True,
                    )
                dst = xqt[:, q8 * 8:(q8 + 1) * 8, :]
                if q8 % 2 == 0:
                    nc.scalar.copy(out=dst, in_=ptr[:])
                else:
                    nc.vector.tensor_copy(out=dst, in_=ptr[:])

        # ---- main fp8 DoubleRow GEMM, streamed over 512-col blocks of N ----
        for nb in range(N_BLKS):
            if nb + 3 < N_BLKS:
                issue_wslab(nb + 3)
            wslab = wslab_tiles[nb]

            for mt in range(M_TILES):
                pm = psum_mm.tile([P, N_BLK], F32, tag="pm")
                for j in range(K_SUPERS):
                    nc.tensor.matmul(
                        out=pm[:],
                        lhsT=xqt_tiles[mt][:, 2 * j:2 * j + 2, :],
                        rhs=wslab[:, 2 * j:2 * j + 2, :],
                        start=(j == 0), stop=(j == K_SUPERS - 1),
                        perf_mode=mybir.MatmulPerfMode.DoubleRow,
                    )
                sb1 = opool.tile([P, N_BLK], F32, tag="sb1")
                nc.scalar.activation(
                    out=sb1[:], in_=pm[:],
                    func=mybir.ActivationFunctionType.Copy, scale=xs_tiles[mt][:],
                )
                sb2 = opool.tile([P, N_BLK], BF16, tag="sb2")
                nc.vector.tensor_mul(sb2[:], sb1[:], ws_b[:, nb * N_BLK:(nb + 1) * N_BLK])
                nc.vector.tensor_add(sb2[:], sb2[:], bias_b[:, nb * N_BLK:(nb + 1) * N_BLK])
                nc.sync.dma_start(
                    out=out_ap[mt * P:(mt + 1) * P, nb * N_BLK:(nb + 1) * N_BLK],
                    in_=sb2[:],
                )

    nc.compile()
    return nc


def _get_program():
    if "nc" not in _PROGRAM_CACHE:
        _PROGRAM_CACHE["nc"] = _build_program()
    return _PROGRAM_CACHE["nc"]


def _run_sharded(x, weight, weight_scales, bias, trace=False):
    x = np.asarray(x).astype(ml_dtypes.bfloat16, copy=False)
    weight = np.asarray(weight, dtype=np.float32)
    weight_scales = np.asarray(weight_scales, dtype=np.float32)
    bias = np.asarray(bias, dtype=np.float32)

    # host-side sharding / layout only:
    # wt[nb, p, ksub, n] = weight[nb*512 + n, ksub*128 + p], re-encoded to
    # fp8 e4m3 (lossless: the reference weights are fp8-round-tripped values)
    wt = np.ascontiguousarray(
        weight.T.reshape(K_SUBS, P, N_BLKS, N_BLK).transpose(2, 1, 0, 3)
    ).astype(ml_dtypes.float8_e4m3)
    in_maps = []
    for c in range(NCORES):
        in_maps.append({
            "x": np.ascontiguousarray(x[c * M_SHARD:(c + 1) * M_SHARD]),
            "wt": wt,
            "ws": weight_scales,
            "bias": bias,
        })

    nc = _get_program()
    res = run_bass_kernel_spmd(nc, in_maps, core_ids=list(range(NCORES)), trace=trace)
    out = np.concatenate([res.results[c]["out"] for c in range(NCORES)], axis=0)
    return out, res.exec_time_ns


def kernel(x, weight, weight_scales, bias):
    out, _ = _run_sharded(x, weight, weight_scales, bias,
                          trace=bool(os.environ.get("KERNEL_TRACE")))
    return out
